# revision 23
# baseline (speedup 1.0000x reference)
"""Trainium2 Bass kernel for nn_ExpertChoiceMoRLayer (Expert-Choice MoR layer).

The reference routes each batch row's S=2048 tokens per recursion r (R=3),
keeps top-K=1024 by router prob in ascending token order, runs a transformer
block (causal attention + SwiGLU MLP) on the selected tokens and scatter-adds
w * block_out into the running total.  All recursions read the ORIGINAL
hidden_states, so the 12 (r, b) blocks are independent.

Device sharding: every (r, b) block splits into two uniform half-shards by
query-token PARITY (even/odd local index).  Both halves compute LN1 and K/V
over all 1024 selected tokens, but Q/attention/Wo/MLP only for their 512
parity tokens, so all 24 shards run one identical SPMD program (3 per core on
8 cores); parity only changes the DATA (host-sliced xQ tensor + causal
boundary mask).

Routing (logits / sigmoid / top-k / sort) runs on host with the same jax ops
as the reference (so selection matches bit-for-bit); gather, scatter-add and
the `w *` weighting fold into the host shard/unshard steps.  That is 0.03% of
the FLOPs; the 464 GFLOP of block compute runs on the NeuronCores in bf16 with
fp32 PSUM accumulation.

On-device layout is feature-major ([feature partition, token free]) end to
end: LN stats via ones-matmul reductions + K=1 ones-matmul partition
broadcasts; scoresT per head via K=64 row-packed matmul pairs (tile_position
(0,0)/(64,0)); softmax denominator fused into the attention matmul as a 65th
ones-column of V; no max-subtraction needed (|scores/8| < ~3 by construction).
"""

import os
import sys

for _p in ("/opt/trn_rl_repo",):
    if _p not in sys.path and os.path.isdir(_p):
        sys.path.insert(0, _p)

import numpy as np
import ml_dtypes

import concourse.bass as bass
import concourse.mybir as mybir
import concourse.tile as tile
from concourse import bacc
from concourse.bass_utils import run_bass_kernel_spmd

BF16 = mybir.dt.bfloat16
F32 = mybir.dt.float32
AF = mybir.ActivationFunctionType
ALU = mybir.AluOpType
BF = ml_dtypes.bfloat16

ALPHA = 0.1
CAPACITY_FACTOR = 0.5
NUM_HEADS = 16
EPS = 1e-6
B, S, H, R = 4, 2048, 1024, 3
I = 4 * H
K = max(1, int(CAPACITY_FACTOR * S))   # 1024 selected tokens
HT = H // 128                          # 8 feature tiles
IT = I // 128                          # 32 intermediate tiles
TQ = K // 2                            # 512 parity query tokens
NSHARD = int(os.environ.get("MOR_NSHARD", "3"))
NCORES = 8

_NC_CACHE = {}
LAST_RESULT = None           # BassKernelResults of the most recent device run
TRACE = bool(int(os.environ.get("MOR_TRACE", "0")))


# ============================================================== device program

def _emit_shard(nc, pools, consts, s):
    sb_acts, sb_w, sb_wd, sb_t, sb_row, sb_rowbf, ps_gemm, ps_sc, ps_at = pools
    ones_col, ones_row, eps_sb = consts

    def dram(name, shape, dt, kind="ExternalInput"):
        return nc.dram_tensor(f"{name}_s{s}", shape, dt, kind=kind)

    xT_d = dram("xT", [HT, 128, K], BF16)
    xQ_d = dram("xQ", [HT, 128, TQ], BF16)
    wq_d = dram("wq", [HT, 128, HT, 128], BF16)   # [mt, kp, kt, mp]
    wk_d = dram("wk", [HT, 128, HT, 128], BF16)
    wv_d = dram("wv", [128, HT, H], BF16)         # [kp, kt, n]
    wo_d = dram("wo", [HT, 128, HT, 128], BF16)
    wg_d = dram("wg", [IT, 128, HT, 128], BF16)
    wu_d = dram("wu", [IT, 128, HT, 128], BF16)
    wd_d = dram("wd", [HT, 128, IT, 128], BF16)
    ln_d = dram("ln", [128, 4, HT], F32)
    mk_d = dram("mk", [128, 64], BF16)
    out_d = dram("outT", [HT, 128, TQ], F32, kind="ExternalOutput")

    # persistent per-shard activations (tags shared across shards -> reuse)
    xT = sb_acts.tile([128, HT, K], BF16, tag="xT")
    hT = sb_acts.tile([128, HT, K], BF16, tag="hT")
    xQ = sb_acts.tile([128, HT, TQ], BF16, tag="xQ")
    hQ = sb_acts.tile([128, HT, TQ], BF16, tag="hQ")
    ln_sb = sb_acts.tile([128, 4, HT], F32, tag="ln")
    mk_sb = sb_acts.tile([128, 64], BF16, tag="mk")

    nc.gpsimd.dma_start(ln_sb[:], ln_d[:, :, :])
    nc.gpsimd.dma_start(mk_sb[:], mk_d[:, :])
    for t in range(HT):
        nc.gpsimd.dma_start(xT[:, t, :], xT_d[t, :, :])
        nc.gpsimd.dma_start(xQ[:, t, :], xQ_d[t, :, :])

    # -------------------------------------------------- layernorm (feat-major)
    def layernorm(src3, ncols, dst3, p_w, p_b):
        for c in range(ncols // 512):
            csl = slice(c * 512, (c + 1) * 512)
            sum_ps = ps_sc.tile([128, 512], F32, tag="scab")
            sq_ps = ps_sc.tile([128, 512], F32, tag="scab")
            for t in range(HT):
                xsq = sb_t.tile([128, 512], BF16, tag="xsq", bufs=4)
                nc.scalar.activation(xsq[:], src3[:, t, csl], AF.Square)
                nc.tensor.matmul(sum_ps[0:1, :], ones_col, src3[:, t, csl],
                                 start=(t == 0), stop=(t == HT - 1))
                nc.tensor.matmul(sq_ps[0:1, :], ones_col, xsq[:],
                                 start=(t == 0), stop=(t == HT - 1))
            mu = sb_row.tile([1, 512], F32, tag="row")
            nc.vector.tensor_scalar_mul(mu[:], sum_ps[0:1, :], 1.0 / H)
            mu2 = sb_row.tile([1, 512], F32, tag="row")
            nc.vector.scalar_tensor_tensor(mu2[:], sum_ps[0:1, :], 1.0 / H,
                                           mu[:], ALU.mult, ALU.mult)
            var = sb_row.tile([1, 512], F32, tag="row")
            nc.vector.scalar_tensor_tensor(var[:], sq_ps[0:1, :], 1.0 / H,
                                           mu2[:], ALU.mult, ALU.subtract)
            nc.scalar.activation(var[:], var[:], AF.Ln, bias=eps_sb)
            rstd = sb_row.tile([1, 512], F32, tag="row")
            nc.scalar.activation(rstd[:], var[:], AF.Exp, scale=-0.5)
            murstd = sb_row.tile([1, 512], F32, tag="row")
            nc.vector.tensor_tensor(murstd[:], mu[:], rstd[:], ALU.mult)
            rstd_bf = sb_rowbf.tile([1, 512], BF16, tag="rowbf")
            murstd_bf = sb_rowbf.tile([1, 512], BF16, tag="rowbf")
            nc.vector.tensor_copy(rstd_bf[:], rstd[:])
            nc.vector.tensor_copy(murstd_bf[:], murstd[:])
            rb_ps = ps_at.tile([128, 512], F32, tag="ata")
            mb_ps = ps_at.tile([128, 512], F32, tag="atb")
            nc.tensor.matmul(rb_ps[:], ones_row, rstd_bf[:], start=True, stop=True)
            nc.tensor.matmul(mb_ps[:], ones_row, murstd_bf[:], start=True, stop=True)
            rb_sb = sb_t.tile([128, 512], BF16, tag="rb_sb")
            mb_sb = sb_t.tile([128, 512], BF16, tag="mb_sb")
            nc.scalar.copy(rb_sb[:], rb_ps[:])
            nc.scalar.copy(mb_sb[:], mb_ps[:])
            for t in range(HT):
                tmp = sb_t.tile([128, 512], BF16, tag="lntmp")
                nc.vector.tensor_tensor(tmp[:], src3[:, t, csl], rb_sb[:], ALU.mult)
                nc.vector.tensor_tensor(tmp[:], tmp[:], mb_sb[:], ALU.subtract)
                nc.scalar.activation(dst3[:, t, csl], tmp[:], AF.Identity,
                                     bias=ln_sb[:, p_b, t:t + 1],
                                     scale=ln_sb[:, p_w, t:t + 1])

    layernorm(xT, K, hT, 0, 1)
    layernorm(xQ, TQ, hQ, 0, 1)

    # -------------------------------------------------------------------- QKV
    kT = sb_acts.tile([128, HT, K], BF16, tag="kT")
    qT = sb_acts.tile([128, HT, TQ], BF16, tag="qT_h2")
    vE = sb_acts.tile([128, HT, NUM_HEADS * 65], BF16, tag="vE_mT")

    for mt in range(HT):
        w_sb = sb_w.tile([128, HT, 128], BF16, tag="wqkv")
        nc.scalar.dma_start(w_sb[:], wq_d[mt, :, :, :])
        q_ps = ps_sc.tile([128, 512], F32, tag="scab")
        for kt in range(HT):
            nc.tensor.matmul(q_ps[:], w_sb[:, kt, :], hQ[:, kt, :],
                             start=(kt == 0), stop=(kt == HT - 1))
        nc.scalar.copy(qT[:, mt, :], q_ps[:])

    for mt in range(HT):
        w_sb = sb_w.tile([128, HT, 128], BF16, tag="wqkv")
        nc.scalar.dma_start(w_sb[:], wk_d[mt, :, :, :])
        for c in range(2):
            k_ps = ps_sc.tile([128, 512], F32, tag="scab")
            for kt in range(HT):
                nc.tensor.matmul(k_ps[:], w_sb[:, kt, :],
                                 hT[:, kt, c * 512:(c + 1) * 512],
                                 start=(kt == 0), stop=(kt == HT - 1))
            nc.scalar.copy(kT[:, mt, c * 512:(c + 1) * 512], k_ps[:])

    for tt in range(HT):
        vrow = vE[:, tt, :].rearrange("p (h c) -> p h c", c=65)
        nc.vector.memset(vrow[:, :, 64], 1.0)          # softmax-denominator ones
    for c in range(2):
        wv_sb = sb_acts.tile([128, HT, 512], BF16, tag="wv")
        nc.scalar.dma_start(wv_sb[:], wv_d[:, :, c * 512:(c + 1) * 512])
        for tt in range(HT):
            vrow = vE[:, tt, :].rearrange("p (h c) -> p h c", c=65)
            v_ps = ps_sc.tile([128, 512], F32, tag="scab")
            for kt in range(HT):
                nc.tensor.matmul(v_ps[:], hT[:, kt, tt * 128:(tt + 1) * 128],
                                 wv_sb[:, kt, :],
                                 start=(kt == 0), stop=(kt == HT - 1))
            nc.scalar.copy(vrow[:, 8 * c:8 * (c + 1), 0:64],
                           v_ps[:].rearrange("p (h d) -> p h d", d=64))

    # -------------------------------------------------------------- attention
    atT = sb_acts.tile([128, HT, TQ], BF16, tag="atT")
    for hp in range(HT):
        ha, hb = 2 * hp, 2 * hp + 1
        at_a = ps_at.tile([128, 512], F32, tag="ata")
        at_b = ps_at.tile([128, 512], F32, tag="atb")
        exps = {}
        for kt in range(HT + 1):
            if kt < HT:
                w = 512 - 64 * kt
                ksl = slice(kt * 128, (kt + 1) * 128)
                sc_ab = ps_sc.tile([128, 1024], F32, tag="scab")
                nc.tensor.matmul(sc_ab[:, 0:w], kT[0:64, hp, ksl],
                                 qT[0:64, hp, 64 * kt:512],
                                 start=True, stop=True, tile_position=(0, 0))
                nc.tensor.matmul(sc_ab[:, 512:512 + w], kT[64:128, hp, ksl],
                                 qT[64:128, hp, 64 * kt:512],
                                 start=True, stop=True, tile_position=(64, 0))
                eab = sb_t.tile([128, 2, 512], BF16, tag="expab", bufs=3)
                sc3 = sc_ab[:].rearrange("p (h w) -> p h w", h=2)
                nc.scalar.activation(eab[:, :, 0:w], sc3[:, :, 0:w],
                                     AF.Exp, scale=0.125)
                nc.vector.tensor_tensor(eab[:, 0, 0:64], eab[:, 0, 0:64],
                                        mk_sb[:], ALU.mult)
                nc.vector.tensor_tensor(eab[:, 1, 0:64], eab[:, 1, 0:64],
                                        mk_sb[:], ALU.mult)
                exps[kt] = eab
            if kt >= 1:
                pk = kt - 1
                pw = 512 - 64 * pk
                eab = exps.pop(pk)
                nc.tensor.matmul(at_a[0:65, 64 * pk:512],
                                 vE[:, pk, ha * 65:(ha + 1) * 65],
                                 eab[:, 0, 0:pw],
                                 start=(pk == 0), stop=(pk == HT - 1))
                nc.tensor.matmul(at_b[0:65, 64 * pk:512],
                                 vE[:, pk, hb * 65:(hb + 1) * 65],
                                 eab[:, 1, 0:pw],
                                 start=(pk == 0), stop=(pk == HT - 1))
        for at_ps, prow in ((at_a, slice(0, 64)), (at_b, slice(64, 128))):
            atU = sb_t.tile([65, 512], F32, tag="atU")
            nc.scalar.copy(atU[:], at_ps[0:65, :])     # frees the PSUM accumulator
            rc = sb_row.tile([1, 512], F32, tag="row")
            nc.vector.reciprocal(rc[:], atU[64:65, :])
            rcb = sb_rowbf.tile([1, 512], BF16, tag="rowbf")
            nc.vector.tensor_copy(rcb[:], rc[:])
            bc_ps = ps_gemm.tile([128, 512], F32, tag="gemm")
            nc.tensor.matmul(bc_ps[0:64, :], ones_row[:, 0:64], rcb[:],
                             start=True, stop=True)
            nc.vector.tensor_tensor(atT[prow, hp, :], atU[0:64, :], bc_ps[0:64, :],
                                    ALU.mult)

    # -------------------------------------------------------- Wo + residual
    x2 = sb_acts.tile([128, HT, TQ], BF16, tag="x2")
    for mt in range(HT):
        w_sb = sb_w.tile([128, HT, 128], BF16, tag="wqkv")
        nc.scalar.dma_start(w_sb[:], wo_d[mt, :, :, :])
        o_ps = ps_gemm.tile([128, 512], F32, tag="gemm")
        for kt in range(HT):
            nc.tensor.matmul(o_ps[:], w_sb[:, kt, :], atT[:, kt, :],
                             start=(kt == 0), stop=(kt == HT - 1))
        o_bf = sb_t.tile([128, 512], BF16, tag="o_bf")
        nc.scalar.copy(o_bf[:], o_ps[:])
        nc.vector.tensor_tensor(x2[:, mt, :], o_bf[:], xQ[:, mt, :], ALU.add)

    # ------------------------------------------------------------------- LN2
    h2 = sb_acts.tile([128, HT, TQ], BF16, tag="qT_h2")
    layernorm(x2, TQ, h2, 2, 3)

    # ------------------------------------------------------------------- MLP
    mT = sb_acts.tile([128, IT, TQ], BF16, tag="vE_mT")
    for mg in range(IT):
        wg_sb = sb_w.tile([128, HT, 128], BF16, tag="wg")
        wu_sb = sb_w.tile([128, HT, 128], BF16, tag="wu")
        nc.sync.dma_start(wg_sb[:], wg_d[mg, :, :, :])
        nc.sync.dma_start(wu_sb[:], wu_d[mg, :, :, :])
        g_ps = ps_gemm.tile([128, 512], F32, tag="gemm")
        for kt in range(HT):
            nc.tensor.matmul(g_ps[:], wg_sb[:, kt, :], h2[:, kt, :],
                             start=(kt == 0), stop=(kt == HT - 1))
        u_ps = ps_gemm.tile([128, 512], F32, tag="gemm")
        for kt in range(HT):
            nc.tensor.matmul(u_ps[:], wu_sb[:, kt, :], h2[:, kt, :],
                             start=(kt == 0), stop=(kt == HT - 1))
        sg = sb_t.tile([128, 512], F32, tag="sg")
        nc.scalar.activation(sg[:], g_ps[:], AF.Silu)
        nc.vector.tensor_tensor(mT[:, mg, :], u_ps[:], sg[:], ALU.mult)

    for md in range(HT):
        wd_sb = sb_wd.tile([128, IT, 128], BF16, tag="wd")
        nc.sync.dma_start(wd_sb[:], wd_d[md, :, :, :])
        d_ps = ps_gemm.tile([128, 512], F32, tag="gemm")
        for kt in range(IT):
            nc.tensor.matmul(d_ps[:], wd_sb[:, kt, :], mT[:, kt, :],
                             start=(kt == 0), stop=(kt == IT - 1))
        d_bf = sb_t.tile([128, 512], BF16, tag="d_bf")
        nc.scalar.copy(d_bf[:], d_ps[:])
        osb = sb_t.tile([128, 512], F32, tag="osb")
        nc.vector.tensor_tensor(osb[:], d_bf[:], x2[:, md, :], ALU.add)
        nc.gpsimd.dma_start(out_d[md, :, :], osb[:])


def build_nc():
    key = NSHARD
    if key in _NC_CACHE:
        return _NC_CACHE[key]
    nc = bacc.Bacc(None, target_bir_lowering=False)
    with tile.TileContext(nc) as tc:
        with (
            tc.tile_pool(name="sb_acts", bufs=1) as sb_acts,
            tc.tile_pool(name="sb_w", bufs=3) as sb_w,
            tc.tile_pool(name="sb_wd", bufs=2) as sb_wd,
            tc.tile_pool(name="sb_t", bufs=2) as sb_t,
            tc.tile_pool(name="sb_row", bufs=5) as sb_row,
            tc.tile_pool(name="sb_rowbf", bufs=2) as sb_rowbf,
            tc.tile_pool(name="sb_c", bufs=1) as sb_c,
            tc.tile_pool(name="ps_gemm", bufs=2, space="PSUM") as ps_gemm,
            tc.tile_pool(name="ps_sc", bufs=2, space="PSUM") as ps_sc,
            tc.tile_pool(name="ps_at", bufs=1, space="PSUM") as ps_at,
        ):
            ones_sb = sb_c.tile([128, 1], BF16, tag="ones_col")
            nc.vector.memset(ones_sb[:], 1.0)
            ones_row = sb_c.tile([1, 128], BF16, tag="ones_row")
            nc.vector.memset(ones_row[:], 1.0)
            eps_sb = sb_c.tile([1, 1], F32, tag="eps")
            nc.vector.memset(eps_sb[:], EPS)
            pools = (sb_acts, sb_w, sb_wd, sb_t, sb_row, sb_rowbf,
                     ps_gemm, ps_sc, ps_at)
            consts = (ones_sb[:], ones_row[:], eps_sb[:])
            for s in range(NSHARD):
                _emit_shard(nc, pools, consts, s)
    nc.compile()
    _NC_CACHE[key] = nc
    return nc


# =============================================================== host program

def _routing(hidden_states, Wr):
    """Mirror the reference's routing ops (same jax primitives/platform) so the
    selected token sets match the reference bit-for-bit."""
    import jax
    import jax.numpy as jnp

    logits_all, idx_all, w_all = [], [], []
    top_k = K
    with jax.default_device(jax.devices("cpu")[0]):
        hs = jnp.asarray(hidden_states)
        for r in range(R):
            logits = hs @ jnp.asarray(Wr[r]).T
            router_probs = jax.nn.sigmoid(logits)[..., 0] * ALPHA
            vals, idx = jax.lax.top_k(router_probs, top_k)
            order = jnp.argsort(idx, axis=-1)
            idx = jnp.take_along_axis(idx, order, axis=-1)
            w = jnp.take_along_axis(vals, order, axis=-1)
            logits_all.append(np.asarray(logits))
            idx_all.append(np.asarray(idx))
            w_all.append(np.asarray(w))
    return (np.stack(logits_all, axis=0),
            np.stack(idx_all, axis=0),
            np.stack(w_all, axis=0))


def _prep_weights_r(Wq, Wk, Wv, Wo, Wg, Wu, Wd, ln1w, ln1b, ln2w, ln2b):
    """Blocked, transposed, bf16 weight layouts for one recursion."""
    def blk_lhst(WT, mt_n, kt_n):
        # WT [K_in, M_out] -> [mt, kp, kt, mp] bf16 blocks for weight-stationary lhsT
        a = WT.reshape(kt_n, 128, mt_n, 128)
        return np.ascontiguousarray(a.transpose(2, 1, 0, 3)).astype(BF)

    out = {}
    out["wq"] = blk_lhst(Wq.T, HT, HT)
    out["wk"] = blk_lhst(Wk.T, HT, HT)
    out["wo"] = blk_lhst(Wo.T, HT, HT)
    out["wg"] = blk_lhst(Wg.T, IT, HT)
    out["wu"] = blk_lhst(Wu.T, IT, HT)
    out["wd"] = blk_lhst(Wd.T, HT, IT)
    wvT = Wv.T.reshape(HT, 128, H)                 # [kt, kp, n]
    out["wv"] = np.ascontiguousarray(wvT.transpose(1, 0, 2)).astype(BF)
    ln = np.stack([ln1w.reshape(HT, 128).T, ln1b.reshape(HT, 128).T,
                   ln2w.reshape(HT, 128).T, ln2b.reshape(HT, 128).T], axis=1)
    out["ln"] = np.ascontiguousarray(ln).astype(np.float32)    # [128, 4, HT]
    return out


def _masks():
    mks = []
    for p in range(2):
        kp = np.arange(128)[:, None]
        c = np.arange(64)[None, :]
        mks.append((2 * c + p >= kp).astype(BF))
    return mks


def shard_inputs(hidden_states, idx, wr_prepped, sid):
    """Build the device input dict for global shard `sid` = ((r*B + b)*2 + p)."""
    p = sid % 2
    b = (sid // 2) % B
    r = sid // (2 * B)
    sel = hidden_states[b][idx[r, b]]                      # [K, H] fp32
    xT = np.ascontiguousarray(sel.T).astype(BF)            # [H, K]
    xQ = np.ascontiguousarray(sel[p::2].T).astype(BF)      # [H, TQ]
    d = {
        "xT": xT.reshape(HT, 128, K),
        "xQ": xQ.reshape(HT, 128, TQ),
        "mk": _MASKS[p],
    }
    d.update(wr_prepped[r])
    return d


_MASKS = _masks()


def kernel(**inputs):
    global LAST_RESULT
    hidden_states = np.asarray(inputs["hidden_states"], dtype=np.float32)
    Wr = np.asarray(inputs["Wr"], dtype=np.float32)

    logits, idx, w = _routing(hidden_states, Wr)

    wr_prepped = [
        _prep_weights_r(inputs["Wq"][r], inputs["Wk"][r], inputs["Wv"][r],
                        inputs["Wo"][r], inputs["Wg"][r], inputs["Wu"][r],
                        inputs["Wd"][r], inputs["ln1w"][r], inputs["ln1b"][r],
                        inputs["ln2w"][r], inputs["ln2b"][r])
        for r in range(R)
    ]

    nc = build_nc()

    nsids = 2 * B * R                       # 24 shards
    assert nsids == NCORES * NSHARD or NSHARD != 3
    in_maps = []
    for c in range(NCORES):
        m = {}
        for slot in range(NSHARD):
            sid = c * NSHARD + slot
            if sid < nsids:
                sh = shard_inputs(hidden_states, idx, wr_prepped, sid)
            else:                            # dummy work for partial builds
                sh = shard_inputs(hidden_states, idx, wr_prepped, 0)
            for k_, v_ in sh.items():
                m[f"{k_}_s{slot}"] = v_
        in_maps.append(m)

    import time as _time
    _t0 = _time.time()
    res = run_bass_kernel_spmd(nc, in_maps, core_ids=list(range(NCORES)),
                               trace=TRACE)
    globals()["LAST_RUN_WALL_S"] = _time.time() - _t0
    LAST_RESULT = res

    total = hidden_states.copy()
    for sid in range(min(nsids, NCORES * NSHARD)):
        p = sid % 2
        b = (sid // 2) % B
        r = sid // (2 * B)
        c, slot = divmod(sid, NSHARD)
        outT = np.asarray(res.results[c][f"outT_s{slot}"], dtype=np.float32)
        out_tok = outT.reshape(H, TQ).T                    # [TQ, H]
        tloc = np.arange(p, K, 2)
        gidx = idx[r, b, tloc]
        total[b, gidx] += w[r, b, tloc][:, None] * out_tok

    return total, logits


# revision 24
# speedup vs baseline: 1.0151x; 1.0151x over previous
"""Trainium2 Bass kernel for nn_ExpertChoiceMoRLayer (Expert-Choice MoR layer).

The reference routes each batch row's S=2048 tokens per recursion r (R=3),
keeps top-K=1024 by router prob in ascending token order, runs a transformer
block (causal attention + SwiGLU MLP) on the selected tokens and scatter-adds
w * block_out into the running total.  All recursions read the ORIGINAL
hidden_states, so the 12 (r, b) blocks are independent.

Device sharding: every (r, b) block splits into two uniform half-shards by
query-token PARITY (even/odd local index).  Both halves compute LN1 and K/V
over all 1024 selected tokens, but Q/attention/Wo/MLP only for their 512
parity tokens, so all 24 shards run one identical SPMD program (3 per core on
8 cores); parity only changes the DATA (host-sliced xQ tensor + causal
boundary mask).

Routing (logits / sigmoid / top-k / sort) runs on host with the same jax ops
as the reference (so selection matches bit-for-bit); gather, scatter-add and
the `w *` weighting fold into the host shard/unshard steps.  That is 0.03% of
the FLOPs; the 464 GFLOP of block compute runs on the NeuronCores in bf16 with
fp32 PSUM accumulation.

On-device layout is feature-major ([feature partition, token free]) end to
end: LN stats via ones-matmul reductions + K=1 ones-matmul partition
broadcasts; scoresT per head via K=64 row-packed matmul pairs (tile_position
(0,0)/(64,0)); softmax denominator fused into the attention matmul as a 65th
ones-column of V; no max-subtraction needed (|scores/8| < ~3 by construction).
"""

import os
import sys

for _p in ("/opt/trn_rl_repo",):
    if _p not in sys.path and os.path.isdir(_p):
        sys.path.insert(0, _p)

import numpy as np
import ml_dtypes

import concourse.bass as bass
import concourse.mybir as mybir
import concourse.tile as tile
from concourse import bacc
from concourse.bass_utils import run_bass_kernel_spmd

BF16 = mybir.dt.bfloat16
F32 = mybir.dt.float32
AF = mybir.ActivationFunctionType
ALU = mybir.AluOpType
BF = ml_dtypes.bfloat16

ALPHA = 0.1
CAPACITY_FACTOR = 0.5
NUM_HEADS = 16
EPS = 1e-6
B, S, H, R = 4, 2048, 1024, 3
I = 4 * H
K = max(1, int(CAPACITY_FACTOR * S))   # 1024 selected tokens
HT = H // 128                          # 8 feature tiles
IT = I // 128                          # 32 intermediate tiles
TQ = K // 2                            # 512 parity query tokens
NSHARD = int(os.environ.get("MOR_NSHARD", "3"))
NCORES = 8

_NC_CACHE = {}
LAST_RESULT = None           # BassKernelResults of the most recent device run
TRACE = bool(int(os.environ.get("MOR_TRACE", "0")))


# ============================================================== device program

def _emit_shard(nc, pools, consts, s):
    sb_acts, sb_w, sb_wd, sb_t, sb_row, sb_rowbf, ps_gemm, ps_sc, ps_at = pools
    ones_col, ones_row, eps_sb = consts

    def dram(name, shape, dt, kind="ExternalInput"):
        return nc.dram_tensor(f"{name}_s{s}", shape, dt, kind=kind)

    xT_d = dram("xT", [HT, 128, K], BF16)
    xQ_d = dram("xQ", [HT, 128, TQ], BF16)
    wq_d = dram("wq", [HT, 128, HT, 128], BF16)   # [mt, kp, kt, mp]
    wk_d = dram("wk", [HT, 128, HT, 128], BF16)
    wv_d = dram("wv", [128, HT, H], BF16)         # [kp, kt, n]
    wo_d = dram("wo", [HT, 128, HT, 128], BF16)
    wg_d = dram("wg", [IT, 128, HT, 128], BF16)
    wu_d = dram("wu", [IT, 128, HT, 128], BF16)
    wd_d = dram("wd", [HT, 128, IT, 128], BF16)
    ln_d = dram("ln", [128, 4, HT], F32)
    mk_d = dram("mk", [128, 64], BF16)
    out_d = dram("outT", [HT, 128, TQ], F32, kind="ExternalOutput")

    # persistent per-shard activations (tags shared across shards -> reuse)
    xT = sb_acts.tile([128, HT, K], BF16, tag="xT")
    hT = sb_acts.tile([128, HT, K], BF16, tag="hT")
    xQ = sb_acts.tile([128, HT, TQ], BF16, tag="xQ")
    hQ = sb_acts.tile([128, HT, TQ], BF16, tag="hQ")
    ln_sb = sb_acts.tile([128, 4, HT], F32, tag="ln")
    mk_sb = sb_acts.tile([128, 64], BF16, tag="mk")

    nc.gpsimd.dma_start(ln_sb[:], ln_d[:, :, :])
    nc.gpsimd.dma_start(mk_sb[:], mk_d[:, :])
    for t in range(HT):
        nc.gpsimd.dma_start(xT[:, t, :], xT_d[t, :, :])
        nc.gpsimd.dma_start(xQ[:, t, :], xQ_d[t, :, :])

    # -------------------------------------------------- layernorm (feat-major)
    def layernorm(src3, ncols, dst3, p_w, p_b):
        for c in range(ncols // 512):
            csl = slice(c * 512, (c + 1) * 512)
            sum_ps = ps_sc.tile([128, 512], F32, tag="scab")
            sq_ps = ps_sc.tile([128, 512], F32, tag="scab")
            for t in range(HT):
                xsq = sb_t.tile([128, 512], BF16, tag="xsq", bufs=4)
                nc.scalar.activation(xsq[:], src3[:, t, csl], AF.Square)
                nc.tensor.matmul(sum_ps[0:1, :], ones_col, src3[:, t, csl],
                                 start=(t == 0), stop=(t == HT - 1))
                nc.tensor.matmul(sq_ps[0:1, :], ones_col, xsq[:],
                                 start=(t == 0), stop=(t == HT - 1))
            mu = sb_row.tile([1, 512], F32, tag="row")
            nc.vector.tensor_scalar_mul(mu[:], sum_ps[0:1, :], 1.0 / H)
            mu2 = sb_row.tile([1, 512], F32, tag="row")
            nc.vector.scalar_tensor_tensor(mu2[:], sum_ps[0:1, :], 1.0 / H,
                                           mu[:], ALU.mult, ALU.mult)
            var = sb_row.tile([1, 512], F32, tag="row")
            nc.vector.scalar_tensor_tensor(var[:], sq_ps[0:1, :], 1.0 / H,
                                           mu2[:], ALU.mult, ALU.subtract)
            nc.scalar.activation(var[:], var[:], AF.Ln, bias=eps_sb)
            rstd = sb_row.tile([1, 512], F32, tag="row")
            nc.scalar.activation(rstd[:], var[:], AF.Exp, scale=-0.5)
            murstd = sb_row.tile([1, 512], F32, tag="row")
            nc.vector.tensor_tensor(murstd[:], mu[:], rstd[:], ALU.mult)
            rstd_bf = sb_rowbf.tile([1, 512], BF16, tag="rowbf")
            murstd_bf = sb_rowbf.tile([1, 512], BF16, tag="rowbf")
            nc.vector.tensor_copy(rstd_bf[:], rstd[:])
            nc.vector.tensor_copy(murstd_bf[:], murstd[:])
            rb_ps = ps_at.tile([128, 512], F32, tag="ata")
            mb_ps = ps_at.tile([128, 512], F32, tag="atb")
            nc.tensor.matmul(rb_ps[:], ones_row, rstd_bf[:], start=True, stop=True)
            nc.tensor.matmul(mb_ps[:], ones_row, murstd_bf[:], start=True, stop=True)
            rb_sb = sb_t.tile([128, 512], BF16, tag="rb_sb")
            mb_sb = sb_t.tile([128, 512], BF16, tag="mb_sb")
            nc.scalar.copy(rb_sb[:], rb_ps[:])
            nc.scalar.copy(mb_sb[:], mb_ps[:])
            for t in range(HT):
                tmp = sb_t.tile([128, 512], BF16, tag="lntmp")
                nc.vector.tensor_tensor(tmp[:], src3[:, t, csl], rb_sb[:], ALU.mult)
                nc.vector.tensor_tensor(tmp[:], tmp[:], mb_sb[:], ALU.subtract)
                nc.scalar.activation(dst3[:, t, csl], tmp[:], AF.Identity,
                                     bias=ln_sb[:, p_b, t:t + 1],
                                     scale=ln_sb[:, p_w, t:t + 1])

    layernorm(xT, K, hT, 0, 1)
    layernorm(xQ, TQ, hQ, 0, 1)

    # -------------------------------------------------------------------- QKV
    kT = sb_acts.tile([128, HT, K], BF16, tag="kT")
    qT = sb_acts.tile([128, HT, TQ], BF16, tag="qT_h2")
    vE = sb_acts.tile([128, HT, NUM_HEADS * 65], BF16, tag="vE_mT")

    for mt in range(HT):
        w_sb = sb_w.tile([128, HT, 128], BF16, tag="wqkv")
        nc.scalar.dma_start(w_sb[:], wq_d[mt, :, :, :])
        q_ps = ps_sc.tile([128, 512], F32, tag="scab")
        for kt in range(HT):
            nc.tensor.matmul(q_ps[:], w_sb[:, kt, :], hQ[:, kt, :],
                             start=(kt == 0), stop=(kt == HT - 1))
        nc.scalar.copy(qT[:, mt, :], q_ps[:])

    for mt in range(HT):
        w_sb = sb_w.tile([128, HT, 128], BF16, tag="wqkv")
        nc.scalar.dma_start(w_sb[:], wk_d[mt, :, :, :])
        for c in range(2):
            k_ps = ps_sc.tile([128, 512], F32, tag="scab")
            for kt in range(HT):
                nc.tensor.matmul(k_ps[:], w_sb[:, kt, :],
                                 hT[:, kt, c * 512:(c + 1) * 512],
                                 start=(kt == 0), stop=(kt == HT - 1))
            nc.scalar.copy(kT[:, mt, c * 512:(c + 1) * 512], k_ps[:])

    for tt in range(HT):
        vrow = vE[:, tt, :].rearrange("p (h c) -> p h c", c=65)
        nc.vector.memset(vrow[:, :, 64], 1.0)          # softmax-denominator ones
    for c in range(2):
        wv_sb = sb_acts.tile([128, HT, 512], BF16, tag="wv")
        nc.scalar.dma_start(wv_sb[:], wv_d[:, :, c * 512:(c + 1) * 512])
        for tt in range(HT):
            vrow = vE[:, tt, :].rearrange("p (h c) -> p h c", c=65)
            v_ps = ps_sc.tile([128, 512], F32, tag="scab")
            for kt in range(HT):
                nc.tensor.matmul(v_ps[:], hT[:, kt, tt * 128:(tt + 1) * 128],
                                 wv_sb[:, kt, :],
                                 start=(kt == 0), stop=(kt == HT - 1))
            nc.scalar.copy(vrow[:, 8 * c:8 * (c + 1), 0:64],
                           v_ps[:].rearrange("p (h d) -> p h d", d=64))

    # -------------------------------------------------------------- attention
    atT = sb_acts.tile([128, HT, TQ], BF16, tag="atT")
    for hp in range(HT):
        ha, hb = 2 * hp, 2 * hp + 1
        at_a = ps_at.tile([128, 512], F32, tag="ata")
        at_b = ps_at.tile([128, 512], F32, tag="atb")
        stages = [(0,), (1,), (2,), (3,), (4, 5), (6, 7)]
        pend = []
        for si in range(len(stages) + 1):
            if si < len(stages):
                kts = stages[si]
                sc_ab = ps_sc.tile([128, 1024], F32, tag="scab")
                eab = sb_t.tile([128, 1024], BF16, tag="expab", bufs=3)
                if len(kts) == 1:
                    kt = kts[0]
                    w = 512 - 64 * kt
                    ksl = slice(kt * 128, (kt + 1) * 128)
                    nc.tensor.matmul(sc_ab[:, 0:w], kT[0:64, hp, ksl],
                                     qT[0:64, hp, 64 * kt:512],
                                     start=True, stop=True, tile_position=(0, 0))
                    nc.tensor.matmul(sc_ab[:, 512:512 + w], kT[64:128, hp, ksl],
                                     qT[64:128, hp, 64 * kt:512],
                                     start=True, stop=True, tile_position=(64, 0))
                    sc3 = sc_ab[:].rearrange("p (s w) -> p s w", s=2)
                    e3 = eab[:].rearrange("p (s w) -> p s w", s=2)
                    nc.scalar.activation(e3[:, :, 0:w], sc3[:, :, 0:w],
                                         AF.Exp, scale=0.125)
                    nc.vector.tensor_tensor(
                        e3[:, :, 0:64], e3[:, :, 0:64],
                        mk_sb[:, None, :].to_broadcast((128, 2, 64)), ALU.mult)
                    pend.append([(kt, e3[:, 0, 0:w], e3[:, 1, 0:w])])
                else:
                    k0, k1 = kts
                    w0, w1 = 512 - 64 * k0, 512 - 64 * k1
                    k0sl = slice(k0 * 128, (k0 + 1) * 128)
                    k1sl = slice(k1 * 128, (k1 + 1) * 128)
                    nc.tensor.matmul(sc_ab[:, 0:w0], kT[0:64, hp, k0sl],
                                     qT[0:64, hp, 64 * k0:512],
                                     start=True, stop=True, tile_position=(0, 0))
                    nc.tensor.matmul(sc_ab[:, 256:256 + w1], kT[0:64, hp, k1sl],
                                     qT[0:64, hp, 64 * k1:512],
                                     start=True, stop=True, tile_position=(0, 0))
                    nc.tensor.matmul(sc_ab[:, 512:512 + w0], kT[64:128, hp, k0sl],
                                     qT[64:128, hp, 64 * k0:512],
                                     start=True, stop=True, tile_position=(64, 0))
                    nc.tensor.matmul(sc_ab[:, 768:768 + w1], kT[64:128, hp, k1sl],
                                     qT[64:128, hp, 64 * k1:512],
                                     start=True, stop=True, tile_position=(64, 0))
                    sc4 = sc_ab[:].rearrange("p (s w) -> p s w", s=4)
                    e4 = eab[:].rearrange("p (s w) -> p s w", s=4)
                    nc.scalar.activation(e4[:, :, 0:w0], sc4[:, :, 0:w0],
                                         AF.Exp, scale=0.125)
                    nc.vector.tensor_tensor(
                        e4[:, :, 0:64], e4[:, :, 0:64],
                        mk_sb[:, None, :].to_broadcast((128, 4, 64)), ALU.mult)
                    pend.append([(k0, e4[:, 0, 0:w0], e4[:, 2, 0:w0]),
                                 (k1, e4[:, 1, 0:w1], e4[:, 3, 0:w1])])
            if si >= 1:
                for kt, ra, rb in pend.pop(0):
                    nc.tensor.matmul(at_a[0:65, 64 * kt:512],
                                     vE[:, kt, ha * 65:(ha + 1) * 65], ra,
                                     start=(kt == 0), stop=(kt == HT - 1))
                    nc.tensor.matmul(at_b[0:65, 64 * kt:512],
                                     vE[:, kt, hb * 65:(hb + 1) * 65], rb,
                                     start=(kt == 0), stop=(kt == HT - 1))
        for at_ps, prow in ((at_a, slice(0, 64)), (at_b, slice(64, 128))):
            atU = sb_t.tile([65, 512], F32, tag="atU")
            nc.scalar.copy(atU[:], at_ps[0:65, :])     # frees the PSUM accumulator
            rc = sb_row.tile([1, 512], F32, tag="row")
            nc.vector.reciprocal(rc[:], atU[64:65, :])
            rcb = sb_rowbf.tile([1, 512], BF16, tag="rowbf")
            nc.vector.tensor_copy(rcb[:], rc[:])
            bc_ps = ps_gemm.tile([128, 512], F32, tag="gemm")
            nc.tensor.matmul(bc_ps[0:64, :], ones_row[:, 0:64], rcb[:],
                             start=True, stop=True)
            nc.vector.tensor_tensor(atT[prow, hp, :], atU[0:64, :], bc_ps[0:64, :],
                                    ALU.mult)

    # -------------------------------------------------------- Wo + residual
    x2 = sb_acts.tile([128, HT, TQ], BF16, tag="x2")
    for mt in range(HT):
        w_sb = sb_w.tile([128, HT, 128], BF16, tag="wqkv")
        nc.scalar.dma_start(w_sb[:], wo_d[mt, :, :, :])
        o_ps = ps_gemm.tile([128, 512], F32, tag="gemm")
        for kt in range(HT):
            nc.tensor.matmul(o_ps[:], w_sb[:, kt, :], atT[:, kt, :],
                             start=(kt == 0), stop=(kt == HT - 1))
        o_bf = sb_t.tile([128, 512], BF16, tag="o_bf")
        nc.scalar.copy(o_bf[:], o_ps[:])
        nc.vector.tensor_tensor(x2[:, mt, :], o_bf[:], xQ[:, mt, :], ALU.add)

    # ------------------------------------------------------------------- LN2
    h2 = sb_acts.tile([128, HT, TQ], BF16, tag="qT_h2")
    layernorm(x2, TQ, h2, 2, 3)

    # ------------------------------------------------------------------- MLP
    mT = sb_acts.tile([128, IT, TQ], BF16, tag="vE_mT")
    for mg in range(IT):
        wg_sb = sb_w.tile([128, HT, 128], BF16, tag="wg")
        wu_sb = sb_w.tile([128, HT, 128], BF16, tag="wu")
        nc.sync.dma_start(wg_sb[:], wg_d[mg, :, :, :])
        nc.sync.dma_start(wu_sb[:], wu_d[mg, :, :, :])
        g_ps = ps_gemm.tile([128, 512], F32, tag="gemm")
        for kt in range(HT):
            nc.tensor.matmul(g_ps[:], wg_sb[:, kt, :], h2[:, kt, :],
                             start=(kt == 0), stop=(kt == HT - 1))
        u_ps = ps_gemm.tile([128, 512], F32, tag="gemm")
        for kt in range(HT):
            nc.tensor.matmul(u_ps[:], wu_sb[:, kt, :], h2[:, kt, :],
                             start=(kt == 0), stop=(kt == HT - 1))
        sg = sb_t.tile([128, 512], F32, tag="sg")
        nc.scalar.activation(sg[:], g_ps[:], AF.Silu)
        nc.vector.tensor_tensor(mT[:, mg, :], u_ps[:], sg[:], ALU.mult)

    for md in range(HT):
        wd_sb = sb_wd.tile([128, IT, 128], BF16, tag="wd")
        nc.sync.dma_start(wd_sb[:], wd_d[md, :, :, :])
        d_ps = ps_gemm.tile([128, 512], F32, tag="gemm")
        for kt in range(IT):
            nc.tensor.matmul(d_ps[:], wd_sb[:, kt, :], mT[:, kt, :],
                             start=(kt == 0), stop=(kt == IT - 1))
        d_bf = sb_t.tile([128, 512], BF16, tag="d_bf")
        nc.scalar.copy(d_bf[:], d_ps[:])
        osb = sb_t.tile([128, 512], F32, tag="osb")
        nc.vector.tensor_tensor(osb[:], d_bf[:], x2[:, md, :], ALU.add)
        nc.gpsimd.dma_start(out_d[md, :, :], osb[:])


def build_nc():
    key = NSHARD
    if key in _NC_CACHE:
        return _NC_CACHE[key]
    nc = bacc.Bacc(None, target_bir_lowering=False)
    with tile.TileContext(nc) as tc:
        with (
            tc.tile_pool(name="sb_acts", bufs=1) as sb_acts,
            tc.tile_pool(name="sb_w", bufs=3) as sb_w,
            tc.tile_pool(name="sb_wd", bufs=2) as sb_wd,
            tc.tile_pool(name="sb_t", bufs=2) as sb_t,
            tc.tile_pool(name="sb_row", bufs=5) as sb_row,
            tc.tile_pool(name="sb_rowbf", bufs=2) as sb_rowbf,
            tc.tile_pool(name="sb_c", bufs=1) as sb_c,
            tc.tile_pool(name="ps_gemm", bufs=2, space="PSUM") as ps_gemm,
            tc.tile_pool(name="ps_sc", bufs=2, space="PSUM") as ps_sc,
            tc.tile_pool(name="ps_at", bufs=1, space="PSUM") as ps_at,
        ):
            ones_sb = sb_c.tile([128, 1], BF16, tag="ones_col")
            nc.vector.memset(ones_sb[:], 1.0)
            ones_row = sb_c.tile([1, 128], BF16, tag="ones_row")
            nc.vector.memset(ones_row[:], 1.0)
            eps_sb = sb_c.tile([1, 1], F32, tag="eps")
            nc.vector.memset(eps_sb[:], EPS)
            pools = (sb_acts, sb_w, sb_wd, sb_t, sb_row, sb_rowbf,
                     ps_gemm, ps_sc, ps_at)
            consts = (ones_sb[:], ones_row[:], eps_sb[:])
            for s in range(NSHARD):
                _emit_shard(nc, pools, consts, s)
    nc.compile()
    _NC_CACHE[key] = nc
    return nc


# =============================================================== host program

def _routing(hidden_states, Wr):
    """Mirror the reference's routing ops (same jax primitives/platform) so the
    selected token sets match the reference bit-for-bit."""
    import jax
    import jax.numpy as jnp

    logits_all, idx_all, w_all = [], [], []
    top_k = K
    with jax.default_device(jax.devices("cpu")[0]):
        hs = jnp.asarray(hidden_states)
        for r in range(R):
            logits = hs @ jnp.asarray(Wr[r]).T
            router_probs = jax.nn.sigmoid(logits)[..., 0] * ALPHA
            vals, idx = jax.lax.top_k(router_probs, top_k)
            order = jnp.argsort(idx, axis=-1)
            idx = jnp.take_along_axis(idx, order, axis=-1)
            w = jnp.take_along_axis(vals, order, axis=-1)
            logits_all.append(np.asarray(logits))
            idx_all.append(np.asarray(idx))
            w_all.append(np.asarray(w))
    return (np.stack(logits_all, axis=0),
            np.stack(idx_all, axis=0),
            np.stack(w_all, axis=0))


def _prep_weights_r(Wq, Wk, Wv, Wo, Wg, Wu, Wd, ln1w, ln1b, ln2w, ln2b):
    """Blocked, transposed, bf16 weight layouts for one recursion."""
    def blk_lhst(WT, mt_n, kt_n):
        # WT [K_in, M_out] -> [mt, kp, kt, mp] bf16 blocks for weight-stationary lhsT
        a = WT.reshape(kt_n, 128, mt_n, 128)
        return np.ascontiguousarray(a.transpose(2, 1, 0, 3)).astype(BF)

    out = {}
    out["wq"] = blk_lhst(Wq.T, HT, HT)
    out["wk"] = blk_lhst(Wk.T, HT, HT)
    out["wo"] = blk_lhst(Wo.T, HT, HT)
    out["wg"] = blk_lhst(Wg.T, IT, HT)
    out["wu"] = blk_lhst(Wu.T, IT, HT)
    out["wd"] = blk_lhst(Wd.T, HT, IT)
    wvT = Wv.T.reshape(HT, 128, H)                 # [kt, kp, n]
    out["wv"] = np.ascontiguousarray(wvT.transpose(1, 0, 2)).astype(BF)
    ln = np.stack([ln1w.reshape(HT, 128).T, ln1b.reshape(HT, 128).T,
                   ln2w.reshape(HT, 128).T, ln2b.reshape(HT, 128).T], axis=1)
    out["ln"] = np.ascontiguousarray(ln).astype(np.float32)    # [128, 4, HT]
    return out


def _masks():
    mks = []
    for p in range(2):
        kp = np.arange(128)[:, None]
        c = np.arange(64)[None, :]
        mks.append((2 * c + p >= kp).astype(BF))
    return mks


def shard_inputs(hidden_states, idx, wr_prepped, sid):
    """Build the device input dict for global shard `sid` = ((r*B + b)*2 + p)."""
    p = sid % 2
    b = (sid // 2) % B
    r = sid // (2 * B)
    sel = hidden_states[b][idx[r, b]]                      # [K, H] fp32
    xT = np.ascontiguousarray(sel.T).astype(BF)            # [H, K]
    xQ = np.ascontiguousarray(sel[p::2].T).astype(BF)      # [H, TQ]
    d = {
        "xT": xT.reshape(HT, 128, K),
        "xQ": xQ.reshape(HT, 128, TQ),
        "mk": _MASKS[p],
    }
    d.update(wr_prepped[r])
    return d


_MASKS = _masks()


def kernel(**inputs):
    global LAST_RESULT
    hidden_states = np.asarray(inputs["hidden_states"], dtype=np.float32)
    Wr = np.asarray(inputs["Wr"], dtype=np.float32)

    logits, idx, w = _routing(hidden_states, Wr)

    wr_prepped = [
        _prep_weights_r(inputs["Wq"][r], inputs["Wk"][r], inputs["Wv"][r],
                        inputs["Wo"][r], inputs["Wg"][r], inputs["Wu"][r],
                        inputs["Wd"][r], inputs["ln1w"][r], inputs["ln1b"][r],
                        inputs["ln2w"][r], inputs["ln2b"][r])
        for r in range(R)
    ]

    nc = build_nc()

    nsids = 2 * B * R                       # 24 shards
    assert nsids == NCORES * NSHARD or NSHARD != 3
    in_maps = []
    for c in range(NCORES):
        m = {}
        for slot in range(NSHARD):
            sid = c * NSHARD + slot
            if sid < nsids:
                sh = shard_inputs(hidden_states, idx, wr_prepped, sid)
            else:                            # dummy work for partial builds
                sh = shard_inputs(hidden_states, idx, wr_prepped, 0)
            for k_, v_ in sh.items():
                m[f"{k_}_s{slot}"] = v_
        in_maps.append(m)

    import time as _time
    _t0 = _time.time()
    res = run_bass_kernel_spmd(nc, in_maps, core_ids=list(range(NCORES)),
                               trace=TRACE)
    globals()["LAST_RUN_WALL_S"] = _time.time() - _t0
    LAST_RESULT = res

    total = hidden_states.copy()
    for sid in range(min(nsids, NCORES * NSHARD)):
        p = sid % 2
        b = (sid // 2) % B
        r = sid // (2 * B)
        c, slot = divmod(sid, NSHARD)
        outT = np.asarray(res.results[c][f"outT_s{slot}"], dtype=np.float32)
        out_tok = outT.reshape(H, TQ).T                    # [TQ, H]
        tloc = np.arange(p, K, 2)
        gidx = idx[r, b, tloc]
        total[b, gidx] += w[r, b, tloc][:, None] * out_tok

    return total, logits
